# revision 1
# baseline (speedup 1.0000x reference)
"""Pointer-generator (CopyModule) kernel for Trainium2, 8 NeuronCores.

Math (per batch b, target row t):
    ctx[t,h]   = sum_s attn[t,s] * enc[s,h]
    p[t]       = sigmoid([ctx; dec] @ W_gen + b_gen)
    Z[t]       = sum_v exp(logits[t,v])            (softmax denom, no max-sub:
                                                    logits ~ N(0,1), exp is safe)
    out[t,v]   = ln(p/Z * exp(logits[t,v]) + (1-p) * C[t,v] + 1e-12)
    C[t,v]     = sum_{s: ids[s]==v} attn[t,s]      (scatter-add, nonzero on
                                                    <=512 vocab columns)

Sharding: B*T_tgt = 1024 rows -> 128 rows per core (= SBUF partitions), the
full vocab V on the free axis. Core c handles batch c//2, t-rows (c%2)*128.

The two big streams travel as fp16: logits stream in fp16 (exp'd on the fly
into a resident fp16 exp buffer), the dense output ln(p/Z*exp + 1e-12)
streams out fp16 (dense |expected| >= 5.6, so fp16's ~2^-11 relative
rounding lands ~1.5e-3 relative error, well under the 2e-2 gate).  The p /
copy path stays fully fp32: copy columns can have |expected| ~ 0, so the
fix tensor must track the reference to ~1e-5.  The scatter is handled
sparsely: a [128, 512] fp32 "fix" tensor holds exact values at the <=512
touched vocab columns (C = attn @ D, with D a host-built 0/1 dedup matrix);
the host writes fix into the touched columns of the dense output (pure
index-addressed data movement; all FLOPs stay on device).

Scheduling: the scalar (ACT) engine is the bottleneck (two 1-elem/cycle
sweeps over V: exp for Z, then ln for the output), so its program is kept
to [exp sweep | tiny sigmoid exp | ln-table load | ln sweep] with the side
packs dispatched mid-sweep on the scalar engine's own HWDGE ring (a second
DMA queue that drains concurrently with the logit stream on the sync ring).
ctx is computed with attn as the matmul weights (8 matmuls of 512-wide
output instead of 32 of 128-wide), and the [ctx; dec] @ W_gen contraction
runs on DVE as three chained tensor_tensor_reduce dots against a W_gen
broadcast built by a rank-1 ones-matmul, so p is ready well before Z.
First logit chunks are small so exp starts ~2us earlier; last write chunks
are small to cut the drain tail.
"""

import os

import numpy as np

import concourse.bass as bass
import concourse.mybir as mybir
import concourse.tile as tile
from concourse import bacc, bass_utils
from concourse.bass import ts

B, T, S, H, V = 4, 256, 512, 1024, 32100
P = 128           # rows per core
NCORES = 8
RCH = [535, 1070, 2140, 4280, 5350, 5350, 5350, 5350, 2675]  # read/exp chunks
WCH = [6420, 6420, 6420, 6420, 3210, 2140, 1070]        # write/ln chunks
KU = 512          # padded width of the unique-column (copy) block
SC = S // P       # 4 s-chunks
HC = H // P       # 8 h-chunks
HB = 512          # h-block width for ctx matmuls (PSUM bank width)
# packA layout (columns, all fp32)
OFF_ATTN = 0                      # [P, SC*P]    attn^T row-chunked
OFF_ENC = OFF_ATTN + SC * P       # [P, SC*H]    enc row-chunked
PKA1 = OFF_ENC + SC * H           # 4608: ctx inputs (packA1)
OFF_DEC = 0                       # [P, HC*P]    dec^T row-chunked
OFF_W = OFF_DEC + HC * P          # [P, 2*HC]    W_gen row-chunked
OFF_LGU = OFF_W + 2 * HC          # [P, KU]      logits at unique columns
PKA2 = OFF_LGU + KU               # 1552: p-tail + fix inputs (packA2)
PKB = SC * KU                     # packB: dedup one-hot row-chunked, 2048
F32 = mybir.dt.float32
F16 = mybir.dt.float16
EPS = 1e-12
AF = mybir.ActivationFunctionType
ALU = mybir.AluOpType

_CACHE: dict = {}
LAST_RESULTS = None  # BassKernelResults of the last run (for test harness)


def _ensure_ntff_hook():
    """Register the axon NTFF profiling hook (the agent image's antenv lacks
    the axon_hooks shim module; rebuild it + the ctypes hook ourselves).
    Only needed for KERNEL_TRACE=1 profiling runs; failures are harmless."""
    try:
        import antenv.axon_hooks  # noqa: F401
        return
    except ImportError:
        pass
    try:
        import sys
        import types

        import antenv
        import importlib.util

        spec = importlib.util.find_spec("trn_agent_boot.trn_boot")
        if spec is None:
            sys.path.insert(0, "/root/.axon_site")
        from trn_agent_boot.trn_boot import _ntff_profile_via_ctypes

        mod = types.ModuleType("antenv.axon_hooks")
        mod._hook = _ntff_profile_via_ctypes("/opt/axon/libaxon_pjrt.so")

        def set_axon_ntff_profile_hook(h):
            mod._hook = h

        def get_axon_ntff_profile_hook():
            return mod._hook

        mod.set_axon_ntff_profile_hook = set_axon_ntff_profile_hook
        mod.get_axon_ntff_profile_hook = get_axon_ntff_profile_hook
        sys.modules["antenv.axon_hooks"] = mod
        antenv.axon_hooks = mod
    except Exception as e:  # pragma: no cover
        print(f"NTFF hook setup failed ({e}); tracing disabled")


def _build(bgen: float):
    nc = bacc.Bacc(
        "TRN2", target_bir_lowering=False, debug=False, num_devices=NCORES
    )

    lg = nc.dram_tensor("lg", [P, V], F16, kind="ExternalInput")
    packa1 = nc.dram_tensor("packa1", [P, PKA1], F32, kind="ExternalInput")
    packa2 = nc.dram_tensor("packa2", [P, PKA2], F32, kind="ExternalInput")
    packb = nc.dram_tensor("packb", [P, PKB], F32, kind="ExternalInput")
    outd = nc.dram_tensor("outd", [P, V], F16, kind="ExternalOutput")
    outf = nc.dram_tensor("outf", [P, KU], F32, kind="ExternalOutput")

    NRC = len(RCH)
    roff = [sum(RCH[:i]) for i in range(NRC)]
    woff = [sum(WCH[:i]) for i in range(len(WCH))]

    with tile.TileContext(nc) as tc:
        with (
            tc.tile_pool(name="const", bufs=1) as cp,
            tc.tile_pool(name="inb", bufs=4) as inp,
            tc.tile_pool(name="outb", bufs=3) as outp,
            tc.tile_pool(name="psc", bufs=2, space="PSUM") as ppc,
            tc.tile_pool(name="ps1", bufs=1, space="PSUM") as pp1,
        ):
            expres = cp.tile([P, V], F16)    # resident exp(logits), fp16
            zparts = cp.tile([P, NRC], F32)
            pka1_sb = cp.tile([P, PKA1], F32)
            pka2_sb = cp.tile([P, PKA2], F32)
            pkb_sb = cp.tile([P, PKB], F32)

            eps_sb = cp.tile([P, 1], F32)
            nc.vector.memset(eps_sb[:], EPS)
            bg_sb = cp.tile([P, 1], F32)
            nc.vector.memset(bg_sb[:], float(bgen))

            # sync ring: W_gen row first (tiny; unblocks the W broadcast),
            # then the logit chunks.  The two side packs ride the scalar
            # engine's HWDGE ring (dispatched mid exp-sweep, below) so they
            # drain concurrently instead of stalling the logit FIFO.
            ibs = {}

            def load(i):
                ib = inp.tile([P, 5350], F16)
                nc.sync.dma_start(
                    out=ib[:, : RCH[i]], in_=lg[:, roff[i] : roff[i] + RCH[i]]
                )
                ibs[i] = ib

            for i in range(NRC):
                load(i)

            # pass 1 on ACT: exp chunks with running row-sum accumulators;
            # pack DMAs dispatched after chunk 3; gexp (copy-path exp) after
            # chunk 5, by which time packA has landed.
            gexp = cp.tile([P, KU], F32)

            def exp_chunk(i):
                nc.scalar.activation(
                    out=expres[:, roff[i] : roff[i] + RCH[i]],
                    in_=ibs[i][:, : RCH[i]],
                    func=AF.Exp,
                    accum_out=zparts[:, i : i + 1],
                )

            for i in range(4):
                exp_chunk(i)
            with tc.tile_wait_until(0.013):
                nc.scalar.dma_start(out=pka1_sb[:], in_=packa1[:])
            with tc.tile_wait_until(0.022):
                nc.scalar.dma_start(out=pka2_sb[:], in_=packa2[:])
            with tc.tile_wait_until(0.036):
                nc.scalar.dma_start(out=pkb_sb[:], in_=packb[:])
            exp_chunk(4)
            exp_chunk(5)
            exp_chunk(6)
            nc.scalar.activation(
                out=gexp[:], in_=pka2_sb[:, OFF_LGU : OFF_LGU + KU], func=AF.Exp
            )
            exp_chunk(7)
            exp_chunk(8)

            def attn_sl(sc_):
                return pka1_sb[:, OFF_ATTN + sc_ * P : OFF_ATTN + (sc_ + 1) * P]

            def enc_sl(sc_, hc_):
                o = OFF_ENC + sc_ * H + hc_ * P
                return pka1_sb[:, o : o + P]

            def dec_sl(c_):
                return pka2_sb[:, OFF_DEC + c_ * P : OFF_DEC + (c_ + 1) * P]

            def w_sl(c_):
                return pka2_sb[:, OFF_W + c_ : OFF_W + c_ + 1]

            def dmat_sl(sc_):
                return pkb_sb[:, sc_ * KU : (sc_ + 1) * KU]


            # ctxT[h, t] = sum_s enc[s, h] * attn[t, s]
            ctxT_sb = cp.tile([P, HC, P], F32)
            for hc in range(HC):
                pctx = ppc.tile([P, P], F32, tag="ctx")
                for sc in range(SC):
                    nc.tensor.matmul(
                        out=pctx[:],
                        lhsT=enc_sl(sc, hc),
                        rhs=attn_sl(sc),
                        start=(sc == 0),
                        stop=(sc == SC - 1),
                    )
                nc.vector.tensor_copy(out=ctxT_sb[:, hc, :], in_=pctx[:])

            # copy path matmul: C = attn @ D (tensor engine, off the
            # critical path -- only the end-of-kernel fix needs it)
            pC = pp1.tile([P, KU], F32, tag="C")
            for sc in range(SC):
                nc.tensor.matmul(
                    out=pC[:],
                    lhsT=attn_sl(sc),
                    rhs=dmat_sl(sc),
                    start=(sc == 0),
                    stop=(sc == SC - 1),
                )

            # p logits: sum_h ctx[t,h] W1[h] + sum_h dec[t,h] W2[h]
            pp_p = pp1.tile([P, 1], F32, tag="p")
            for c in range(HC):
                nc.tensor.matmul(
                    out=pp_p[:],
                    lhsT=ctxT_sb[:, c, :],
                    rhs=w_sl(c),
                    start=(c == 0),
                    stop=False,
                )
            for c in range(HC):
                nc.tensor.matmul(
                    out=pp_p[:],
                    lhsT=dec_sl(c),
                    rhs=w_sl(HC + c),
                    start=False,
                    stop=(c == HC - 1),
                )
            xs = cp.tile([P, 1], F32)
            nc.vector.tensor_scalar_add(
                out=xs[:], in0=pp_p[:], scalar1=bg_sb[:, :1]
            )

            # sigmoid computed XLA-style (exp-based, no LUT-sigmoid) so that
            # p and 1-p keep full relative precision in both saturation
            # tails.  All on DVE except the one exp().
            ones = nc.const_aps.tensor(1.0, (P, 1))
            nx = cp.tile([P, 1], F32)   # -x
            nc.vector.tensor_scalar(
                out=nx[:], in0=xs[:], scalar1=-1.0, scalar2=None, op0=ALU.mult
            )
            ax = cp.tile([P, 1], F32)   # |x| = max(x, -x)
            nc.vector.tensor_tensor(
                out=ax[:], in0=xs[:], in1=nx[:], op=ALU.max
            )
            e1 = cp.tile([P, 1], F32)   # exp(-|x|)
            nc.scalar.activation(out=e1[:], in_=ax[:], func=AF.Exp, scale=-1.0)
            den = cp.tile([P, 1], F32)  # 1 + e
            nc.vector.tensor_scalar_add(out=den[:], in0=e1[:], scalar1=1.0)
            rr = cp.tile([P, 1], F32)   # 1/(1+e)
            nc.vector.reciprocal(out=rr[:], in_=den[:])
            er = cp.tile([P, 1], F32)   # e/(1+e)
            nc.vector.tensor_mul(out=er[:], in0=e1[:], in1=rr[:])
            msk = cp.tile([P, 1], mybir.dt.uint8)  # x >= 0
            nc.vector.tensor_scalar(
                out=msk[:], in0=xs[:], scalar1=0.0, scalar2=None, op0=ALU.is_ge
            )
            p_col = cp.tile([P, 1], F32)
            nc.vector.select(
                out=p_col[:], mask=msk[:], on_true=rr[:], on_false=er[:]
            )
            omp = cp.tile([P, 1], F32)  # 1 - p, with p's fp32 rounding
            nc.vector.scalar_tensor_tensor(
                out=omp[:], in0=p_col[:], scalar=-1.0, in1=ones,
                op0=ALU.mult, op1=ALU.add,
            )

            # barrier: Z -> 1/Z -> p/Z
            zsum = cp.tile([P, 1], F32)
            nc.vector.tensor_reduce(
                out=zsum[:], in_=zparts[:], axis=mybir.AxisListType.X, op=ALU.add
            )
            rz = cp.tile([P, 1], F32)
            nc.vector.reciprocal(out=rz[:], in_=zsum[:])
            pz = cp.tile([P, 1], F32)
            nc.vector.tensor_mul(out=pz[:], in0=p_col[:], in1=rz[:])

            # dense pass 2: ln(pz*exp + eps), fp16 out, stream out
            for j, wj in enumerate(WCH):
                ob = outp.tile([P, 6420], F16)
                nc.scalar.activation(
                    out=ob[:, :wj],
                    in_=expres[:, woff[j] : woff[j] + wj],
                    func=AF.Ln,
                    scale=pz[:, :1],
                    bias=eps_sb[:, :1],
                )
                nc.sync.dma_start(out=outd[:, woff[j] : woff[j] + wj], in_=ob[:, :wj])

            # fix values at touched columns: ln(pz*gexp + (1-p)*C + eps)
            cs = cp.tile([P, KU], F32)
            nc.vector.tensor_scalar(
                out=cs[:], in0=pC[:], scalar1=omp[:, :1], scalar2=None,
                op0=ALU.mult,
            )
            fx = cp.tile([P, KU], F32)
            nc.vector.scalar_tensor_tensor(
                out=fx[:],
                in0=gexp[:],
                scalar=pz[:, :1],
                in1=cs[:],
                op0=ALU.mult,
                op1=ALU.add,
            )
            nc.scalar.activation(out=fx[:], in_=fx[:], func=AF.Ln, bias=eps_sb[:, :1])
            nc.scalar.dma_start(out=outf[:], in_=fx[:])

    nc.compile()
    return nc


def _make_packs(attn_b, enc_b, decT_b, D, lgu, wgen):
    """Interleave side tensors so each is a contiguous [128, k] DMA on device
    (row p holds the p-th of every 128-row chunk)."""
    pa1 = np.empty((P, PKA1), np.float32)
    pa1[:, OFF_ATTN:OFF_ENC] = (
        attn_b.T.reshape(SC, P, P).transpose(1, 0, 2).reshape(P, SC * P)
    )
    pa1[:, OFF_ENC:] = (
        enc_b.reshape(SC, P, H).transpose(1, 0, 2).reshape(P, SC * H)
    )
    pa2 = np.empty((P, PKA2), np.float32)
    pa2[:, OFF_DEC:OFF_W] = (
        decT_b.reshape(HC, P, P).transpose(1, 0, 2).reshape(P, HC * P)
    )
    pa2[:, OFF_W:OFF_LGU] = wgen.reshape(2 * HC, P).T
    pa2[:, OFF_LGU:] = lgu
    pb = np.ascontiguousarray(
        D.reshape(SC, P, KU).transpose(1, 0, 2).reshape(P, SC * KU)
    )
    return pa1, pa2, pb


def kernel(**inputs) -> np.ndarray:
    global LAST_RESULTS
    dec = np.asarray(inputs["decoder_hidden_states"], dtype=np.float32)
    attn = np.asarray(inputs["cross_attention_weights"], dtype=np.float32)
    enc = np.asarray(inputs["encoder_hidden_states"], dtype=np.float32)
    logits = np.asarray(inputs["vocab_logits"], dtype=np.float32)
    wgen = np.asarray(inputs["W_gen"], dtype=np.float32)
    bgen = float(np.asarray(inputs["b_gen"]).reshape(-1)[0])
    ids = np.asarray(inputs["source_ids"]).astype(np.int64)

    key = bgen
    nc = _CACHE.get(key)
    if nc is None:
        nc = _build(bgen)
        _CACHE[key] = nc

    uniqs = []
    in_maps = []
    for core in range(NCORES):
        b, half = divmod(core, T // P)
        t0 = half * P
        u, inv = np.unique(ids[b], return_inverse=True)
        D = np.zeros((S, KU), np.float32)
        D[np.arange(S), inv] = 1.0
        lgu = np.zeros((P, KU), np.float32)
        lgu[:, : len(u)] = logits[b, t0 : t0 + P][:, u]
        pa1, pa2, pb = _make_packs(
            attn[b, t0 : t0 + P], enc[b], dec[b, t0 : t0 + P].T, D, lgu, wgen
        )
        in_maps.append(
            {
                "lg": logits[b, t0 : t0 + P].astype(np.float16),
                "packa1": pa1,
                "packa2": pa2,
                "packb": pb,
            }
        )
        uniqs.append(u)

    trace = bool(os.environ.get("KERNEL_TRACE"))
    if trace:
        _ensure_ntff_hook()
    res = bass_utils.run_bass_kernel_spmd(
        nc,
        in_maps,
        core_ids=list(range(NCORES)),
        trace=trace,
    )
    LAST_RESULTS = res

    out = np.empty((B, T, V), np.float32)
    for core in range(NCORES):
        b, half = divmod(core, T // P)
        t0 = half * P
        r = res.results[core]
        out[b, t0 : t0 + P] = r["outd"].astype(np.float32)
        u = uniqs[core]
        out[b, t0 : t0 + P, :][:, u] = r["outf"][:, : len(u)]
    return out

